# revision 8
# baseline (speedup 1.0000x reference)
"""Trainium2 Bass kernel for a 2-layer GCN (segment-sum aggregation).

out = softmax( A @ relu(A @ h @ W1 + b1) @ W2 + b2 ),  A = adjacency (+self loops)

Strategy (8 NeuronCores, node/data parallel):
  - Nodes sharded by range: core k owns nodes [k*12500, (k+1)*12500).
  - Edges routed (on host) to the core owning their dst node, grouped by
    (128-node dst window, src%4 phase). Per-(window,phase) slots are padded
    to the max count over the 8 cores so one SPMD program serves all cores.
  - Layer 1 on device: dma_gather of h[src] rows (256B) from an HBM table
    viewed as [N/4, 1024B] super-rows (dodges the signed-int16 index limit);
    per 128-edge chunk a one-hot [edge x node] matrix is built on the
    VectorEngine (is_equal vs iota) and the TensorEngine accumulates
    aggT = sum featsT @ onehot into PSUM per window (segment-sum).
    Self loops are a dense add.  Then x1T = relu(W1.T @ aggT + b1) and
    y = x1T.T @ W2 (transform BEFORE layer-2 aggregation: 128 -> 40 dims).
  - AllGather of per-core y slices -> full y table (same super-row layout).
  - Layer 2: same gathers/one-hots vs the y table, orientation flipped to
    give node-major [128,64] windows; + self loop + b2; softmax on chip.
"""

import math
import numpy as np

D = 64          # input feature dim (one gather row = 256B)
HID = 128
C = 40
CORES = 8
WIN = 128       # dst window (nodes per one-hot matmul window)
NPHASE = 4      # src mod-4 phases (int16 gather index reach)
GROUP = 4       # windows per gather instruction group


# ----------------------------------------------------------------------------
# Host-side routing
# ----------------------------------------------------------------------------

def route_edges(src, dst, n_nodes, cores=CORES):
    """Group edges by (core, window, phase) into padded static slots.

    Returns dict with static slot table S [NW, NPHASE] (python ints, shared by
    all cores), and per-core int16 index streams / fp32 dst streams.
    """
    nloc = n_nodes // cores
    nw = math.ceil(nloc / WIN)
    src = src.astype(np.int64)
    dst = dst.astype(np.int64)
    core = dst // nloc
    dloc = dst % nloc
    w = dloc // WIN
    phi = src % NPHASE
    key = (w * NPHASE + phi).astype(np.int64)   # per-core key in [0, nw*4)

    counts = np.zeros((cores, nw * NPHASE), np.int64)
    for k in range(cores):
        counts[k] = np.bincount(key[core == k], minlength=nw * NPHASE)
    nmax = counts.max(axis=0)                   # [nw*4]
    S = ((nmax + WIN - 1) // WIN * WIN).astype(np.int64)  # slot sizes, %128
    # ensure every window has at least one chunk (avoids empty PSUM groups)
    for wi in range(nw):
        if S[wi * NPHASE:(wi + 1) * NPHASE].sum() == 0:
            S[wi * NPHASE] = WIN

    # stream order: for group g: for phi: for w in group: slot(w, phi)
    ngroups = math.ceil(nw / GROUP)
    order = []                                  # flat slot order: (w, phi)
    for g in range(ngroups):
        ws = range(g * GROUP, min((g + 1) * GROUP, nw))
        for p in range(NPHASE):
            for wi in ws:
                order.append(wi * NPHASE + p)
    order = np.array(order, np.int64)
    offs = np.zeros(nw * NPHASE, np.int64)      # slot -> stream offset
    offs[order] = np.r_[0, np.cumsum(S[order])[:-1]]
    tot = int(S.sum())

    idx_streams, dst_streams = [], []
    for k in range(cores):
        sel = core == k
        kk = key[sel]
        sidx = np.argsort(kk, kind="stable")
        kk_s = kk[sidx]
        # occurrence rank within each key group
        occ = np.arange(len(kk_s)) - np.repeat(
            np.r_[0, np.cumsum(np.bincount(kk_s, minlength=nw * NPHASE))[:-1]][kk_s], 1)
        pos = offs[kk_s] + occ
        idx = np.zeros(tot, np.int16)           # pad: super-row 0 (valid)
        dsl = np.full(tot, -1000.0, np.float32)  # pad: no one-hot match
        idx[pos] = (src[sel][sidx] >> 2).astype(np.int16)
        dsl[pos] = (dloc[sel][sidx] % WIN).astype(np.float32)
        idx_streams.append(idx)
        dst_streams.append(dsl)
    return dict(S=S, offs=offs, tot=tot, nw=nw, nloc=nloc,
                ngroups=ngroups, idx=idx_streams, dst=dst_streams)


# ----------------------------------------------------------------------------
# Bass program
# ----------------------------------------------------------------------------

def build_program(n_nodes, rt, do_cc=True, l2_table_y=True, maxidx=1024,
                  scratch=16384, skip_compute=False, skip_gather=False):
    import concourse.bass as bass
    import concourse.mybir as mybir
    import concourse.bacc as bacc
    from concourse import tile

    f32 = mybir.dt.float32
    bf16 = mybir.dt.float16
    i16 = mybir.dt.int16
    S, offs, tot, nw, nloc, ngroups = (rt["S"], rt["offs"], rt["tot"],
                                       rt["nw"], rt["nloc"], rt["ngroups"])
    nch = tot // WIN                       # total chunks
    nsup = n_nodes // NPHASE               # super-rows in gather tables
    nlocp = nw * WIN                       # padded local node count
    last_rows = nloc - (nw - 1) * WIN      # rows in the last window

    nc = bacc.Bacc(None, target_bir_lowering=False, debug=False,
                   num_swdge_queues=4, dynamic_dma_scratch_size=scratch)

    h4 = nc.declare_dram_parameter("h4", [nsup, NPHASE * D], bf16, False)
    hTo = nc.declare_dram_parameter("hTo", [D, nlocp], f32, False)
    W1d = nc.declare_dram_parameter("W1", [D, HID], f32, False)
    b1d = nc.declare_dram_parameter("b1", [HID, 1], f32, False)
    W2d = nc.declare_dram_parameter("W2p", [HID, D], f32, False)
    b2d = nc.declare_dram_parameter("b2b", [WIN, D], f32, False)
    idxd = nc.declare_dram_parameter("idx", [128, tot // 16], i16, False)
    dstd = nc.declare_dram_parameter("dstf", [WIN, nch], f32, False)
    iotad = nc.declare_dram_parameter("iota", [WIN, WIN], bf16, False)
    outd = nc.declare_dram_parameter("out", [nloc, C], f32, True)

    cc_in = nc.dram_tensor("cc_in", [nloc, D], bf16)
    y4 = nc.dram_tensor("y4", [CORES * nloc // NPHASE, NPHASE * D], bf16,
                        addr_space="Shared")

    # slot geometry helpers -------------------------------------------------
    def group_windows(g):
        return range(g * GROUP, min((g + 1) * GROUP, nw))

    # per-(group) chunk layout inside the group's gather buffer
    gbase = {}     # (w, phi) -> (group, chunk col within group buffer)
    gchunks = []   # chunks per group
    for g in range(ngroups):
        col = 0
        for p in range(NPHASE):
            for wi in group_windows(g):
                gbase[(wi, p)] = (g, col)
                col += int(S[wi * NPHASE + p]) // WIN
        gchunks.append(col)

    Relu = mybir.ActivationFunctionType.Relu
    Exp = mybir.ActivationFunctionType.Exp
    add_op = mybir.AluOpType.add
    eq_op = mybir.AluOpType.is_equal

    with tile.TileContext(nc) as tc:
        import contextlib
        with contextlib.ExitStack() as ctx:
            cpool = ctx.enter_context(tc.tile_pool(name="const", bufs=1))
            ypool = ctx.enter_context(tc.tile_pool(name="yown", bufs=1))

            idx_sb = cpool.tile([128, tot // 16], i16)
            dst_sb = cpool.tile([WIN, nch], f32)
            iota_sb = cpool.tile([WIN, WIN], bf16)
            hTo_sb = cpool.tile([D, nlocp], f32)
            W1_sb = cpool.tile([D, HID], f32)
            b1_sb = cpool.tile([HID, 1], f32)
            W2_sb = cpool.tile([HID, D], f32)
            b2_sb = cpool.tile([WIN, D], f32)
            yown = ypool.tile([WIN, nw * D], f32)

            nc.sync.dma_start(idx_sb[:], idxd[:])
            nc.sync.dma_start(dst_sb[:], dstd[:])
            nc.sync.dma_start(iota_sb[:], iotad[:])
            nc.sync.dma_start(hTo_sb[:], hTo[:])
            nc.sync.dma_start(W1_sb[:], W1d[:])
            nc.sync.dma_start(b1_sb[:], b1d[:])
            nc.sync.dma_start(W2_sb[:], W2d[:])
            nc.sync.dma_start(b2_sb[:], b2d[:])

            MAXIDX = maxidx  # default 1024: 64 desc/engine x 16 engines/packet
            qctr = [0]      # round-robin SWDGE queue (4 Q7 core pairs)

            def issue_gathers(g, gt, table):
                if skip_gather:
                    return
                for p in range(NPHASE):
                    ws = list(group_windows(g))
                    n = int(sum(S[wi * NPHASE + p] for wi in ws))
                    if n == 0:
                        continue
                    o = int(offs[ws[0] * NPHASE + p])
                    _, col0 = gbase[(ws[0], p)]
                    j = p >> 1          # node-pair within the 4-node super-row
                    for s0 in range(0, n, MAXIDX):
                        ni = min(MAXIDX, n - s0)
                        c0 = col0 + s0 // WIN
                        oo = o + s0
                        nc.gpsimd.dma_gather(
                            out_ap=gt[:, c0 * 2 * D:(c0 + ni // WIN) * 2 * D]
                                .rearrange("p (c f) -> p c f", f=2 * D),
                            in_ap=table[:, j * 2 * D:(j + 1) * 2 * D],
                            idxs_ap=idx_sb[:, oo // 16: (oo + ni) // 16],
                            num_idxs=ni,
                            num_idxs_reg=ni,
                            elem_size=2 * D,
                            elem_step=NPHASE * D,
                            queue_num=qctr[0] % 4,
                        )
                        qctr[0] += 1

            def window_chunks(wi):
                res = []
                for p in range(NPHASE):
                    g, col = gbase[(wi, p)]
                    for c in range(int(S[wi * NPHASE + p]) // WIN):
                        res.append((int(offs[wi * NPHASE + p]) // WIN + c,
                                    col + c, p & 1))
                return res

            # ---------------- stage A: layer 1 ----------------
            with contextlib.ExitStack() as sa:
                gpool = sa.enter_context(tc.tile_pool(name="gatherA", bufs=3))
                ohpool = sa.enter_context(tc.tile_pool(name="ohA", bufs=16))
                aggpool = sa.enter_context(tc.tile_pool(name="aggT", bufs=4))
                xpool = sa.enter_context(tc.tile_pool(name="x1", bufs=4))
                psA = sa.enter_context(
                    tc.tile_pool(name="psA", bufs=3, space="PSUM"))
                psB = sa.enter_context(
                    tc.tile_pool(name="psB", bufs=2, space="PSUM"))
                psC = sa.enter_context(
                    tc.tile_pool(name="psC", bufs=2, space="PSUM"))

                for g in range(ngroups):
                    gt = gpool.tile([WIN, gchunks[g] * 2 * D], bf16, tag="gbuf")
                    issue_gathers(g, gt, h4)
                    for wi in group_windows(g):
                        if skip_compute:
                            ybf = xpool.tile([WIN, D], bf16, tag="ybf")
                            nc.scalar.copy(ybf[:], b2_sb[:])
                            nc.scalar.copy(yown[:, wi * D:(wi + 1) * D],
                                           b2_sb[:])
                            rows = last_rows if wi == nw - 1 else WIN
                            nc.sync.dma_start(
                                cc_in[wi * WIN: wi * WIN + rows, :],
                                ybf[:rows, :])
                            continue
                        chunks = window_chunks(wi)
                        ps = psA.tile([D, WIN], f32)
                        for i, (gcol, lcol, half) in enumerate(chunks):
                            oh = ohpool.tile([WIN, WIN], bf16)
                            nc.vector.tensor_scalar(
                                oh[:], iota_sb[:], dst_sb[:, gcol:gcol + 1],
                                None, eq_op)
                            c0 = lcol * 2 * D + half * D
                            nc.tensor.matmul(
                                ps[:], gt[:, c0:c0 + D], oh[:],
                                start=(i == 0), stop=(i == len(chunks) - 1))
                        aggT = aggpool.tile([D, WIN], f32)
                        nc.vector.tensor_tensor(
                            aggT[:], ps[:], hTo_sb[:, wi * WIN:(wi + 1) * WIN],
                            add_op)
                        ps2 = psB.tile([HID, WIN], f32)
                        nc.tensor.matmul(ps2[:], W1_sb[:], aggT[:])
                        x1 = xpool.tile([HID, WIN], f32)
                        nc.scalar.activation(x1[:], ps2[:], Relu,
                                             bias=b1_sb[:, 0:1])
                        ps3 = psC.tile([WIN, D], f32)
                        nc.tensor.matmul(ps3[:], x1[:], W2_sb[:])
                        nc.scalar.copy(yown[:, wi * D:(wi + 1) * D], ps3[:])
                        ybf = xpool.tile([WIN, D], bf16, tag="ybf")
                        nc.scalar.copy(ybf[:], ps3[:])
                        rows = last_rows if wi == nw - 1 else WIN
                        nc.sync.dma_start(
                            cc_in[wi * WIN: wi * WIN + rows, :],
                            ybf[:rows, :])

            # ---------------- all-gather of y ----------------
            if do_cc:
                nc.gpsimd.collective_compute(
                    "AllGather", mybir.AluOpType.bypass,
                    replica_groups=[list(range(CORES))],
                    ins=[cc_in.ap().opt()], outs=[y4.ap().opt()])

            # ---------------- stage C: layer 2 ----------------
            with contextlib.ExitStack() as sc:
                gpool = sc.enter_context(tc.tile_pool(name="gatherC", bufs=3))
                ohpool = sc.enter_context(tc.tile_pool(name="ohC", bufs=16))
                spool = sc.enter_context(tc.tile_pool(name="smax", bufs=4))
                opool = sc.enter_context(tc.tile_pool(name="outp", bufs=3))
                psD = sc.enter_context(
                    tc.tile_pool(name="psD", bufs=4, space="PSUM"))

                for g in range(ngroups):
                    gt = gpool.tile([WIN, gchunks[g] * 2 * D], bf16, tag="gbufC")
                    issue_gathers(g, gt, y4 if l2_table_y else h4)
                    for wi in group_windows(g):
                        if skip_compute:
                            o = opool.tile([WIN, C], f32)
                            nc.scalar.copy(o[:], b2_sb[:, :C])
                            rows = last_rows if wi == nw - 1 else WIN
                            nc.sync.dma_start(
                                outd[wi * WIN: wi * WIN + rows, :],
                                o[:rows, :])
                            continue
                        chunks = window_chunks(wi)
                        ps = psD.tile([WIN, D], f32)
                        for i, (gcol, lcol, half) in enumerate(chunks):
                            oh = ohpool.tile([WIN, WIN], bf16)
                            nc.vector.tensor_scalar(
                                oh[:], iota_sb[:], dst_sb[:, gcol:gcol + 1],
                                None, eq_op)
                            c0 = lcol * 2 * D + half * D
                            nc.tensor.matmul(
                                ps[:], oh[:], gt[:, c0:c0 + D],
                                start=(i == 0), stop=(i == len(chunks) - 1))
                        t1 = spool.tile([WIN, D], f32, tag="t1")
                        nc.vector.tensor_tensor(
                            t1[:], ps[:], yown[:, wi * D:(wi + 1) * D], add_op)
                        t2 = spool.tile([WIN, D], f32, tag="t2")
                        nc.vector.tensor_tensor(t2[:], t1[:], b2_sb[:], add_op)
                        mx = spool.tile([WIN, 1], f32, tag="mx")
                        nc.vector.tensor_reduce(
                            mx[:], t2[:, :C], mybir.AxisListType.X,
                            mybir.AluOpType.max, negate=True)
                        e = spool.tile([WIN, C], f32, tag="e")
                        nc.scalar.activation(e[:], t2[:, :C], Exp,
                                             bias=mx[:, 0:1])
                        sm = spool.tile([WIN, 1], f32, tag="sm")
                        nc.vector.tensor_reduce(
                            sm[:], e[:], mybir.AxisListType.X, add_op)
                        ri = spool.tile([WIN, 1], f32, tag="ri")
                        nc.vector.reciprocal(ri[:], sm[:])
                        o = opool.tile([WIN, C], f32)
                        nc.vector.tensor_scalar_mul(o[:], e[:], ri[:, 0:1])
                        rows = last_rows if wi == nw - 1 else WIN
                        nc.sync.dma_start(
                            outd[wi * WIN: wi * WIN + rows, :], o[:rows, :])

    nc.finalize()
    return nc


# ----------------------------------------------------------------------------
# Entry point
# ----------------------------------------------------------------------------

def _prepare_inputs(node_embeddings, adjacency_lists, W1, b1, W2, b2, rt):
    n, d = node_embeddings.shape
    nloc, nw = rt["nloc"], rt["nw"]
    nlocp = nw * WIN
    bf = np.float16
    h = np.ascontiguousarray(node_embeddings, np.float32)
    h4 = h.astype(bf).reshape(n // NPHASE, NPHASE * d)
    W2p = np.zeros((HID, D), np.float32)
    W2p[:, :C] = W2
    b2b = np.tile(np.pad(b2.astype(np.float32), (0, D - C)), (WIN, 1))
    iota = np.tile(np.arange(WIN, dtype=np.float32), (WIN, 1))
    in_maps = []
    for k in range(CORES):
        hTo = np.zeros((d, nlocp), np.float32)
        hTo[:, :nloc] = h[k * nloc:(k + 1) * nloc].T
        in_maps.append({
            "h4": h4,
            "hTo": hTo,
            "W1": np.ascontiguousarray(W1, np.float32),
            "b1": np.ascontiguousarray(b1, np.float32).reshape(HID, 1),
            "W2p": W2p,
            "b2b": b2b,
            "idx": np.tile(rt["idx"][k].reshape(-1, 16).T, (8, 1)).copy(),
            "dstf": np.ascontiguousarray(
                rt["dst"][k].reshape(-1, WIN).T),
            "iota": iota.astype(bf),
            "out": np.zeros((nloc, C), np.float32),
        })
    return in_maps


_CACHE = {}


def _get_program(n_nodes, rt_sig, rt):
    key = (n_nodes, rt_sig)
    if key not in _CACHE:
        _CACHE[key] = build_program(n_nodes, rt)
    return _CACHE[key]


def build_all(node_embeddings, adjacency_lists, W1, b1, W2, b2,
              cache=True, **build_flags):
    """Route edges, build (cached) program, prepare per-core inputs."""
    n = node_embeddings.shape[0]
    src = np.asarray(adjacency_lists)[:, 0]
    dst = np.asarray(adjacency_lists)[:, 1]
    rt = route_edges(src, dst, n)
    if cache and not build_flags:
        rt_sig = (rt["tot"], tuple(rt["S"].tolist()))
        nc = _get_program(n, rt_sig, rt)
    else:
        nc = build_program(n, rt, **build_flags)
    in_maps = _prepare_inputs(node_embeddings, adjacency_lists,
                              W1, b1, W2, b2, rt)
    return nc, in_maps, rt


def kernel(node_embeddings, adjacency_lists, W1, b1, W2, b2, trace=False):
    import sys
    if "/opt/trn_rl_repo" not in sys.path:
        sys.path.insert(0, "/opt/trn_rl_repo")
    from concourse import bass_utils

    nc, in_maps, rt = build_all(node_embeddings, adjacency_lists,
                                W1, b1, W2, b2)
    res = bass_utils.run_bass_kernel_spmd(
        nc, in_maps, core_ids=list(range(CORES)), trace=trace)
    out = np.concatenate([res.results[k]["out"] for k in range(CORES)], axis=0)
    kernel.last_result = res
    kernel.last_nc = nc
    kernel.last_in_maps = in_maps
    return out

